# revision 16
# baseline (speedup 1.0000x reference)
"""Single-head attention on 8 Trainium2 NeuronCores.

Sharding: core c handles batch b = c//2, query half h = c%2 (2048 queries,
all 4096 keys). Host passes x^T in bf16 with each core's own query tokens
reordered to columns 0..2047 so the SPMD program is identical on all cores
(attention is permutation-invariant over keys). x^T is additionally
host-swizzled to token-block-major layout so each 512-token block is one
contiguous [128, 4096] DMA — projections start as soon as block 0 lands.

Device pipeline per core:
  1. Per token block tb: pass1 with stationary [Wv|Wk] -> VK2T (V^T rows
     0-63, K^T rows 64-127, single fused bias-add), K^T duplicated to a
     [64, S] tile via DVE copy; pass2 [Wq|Wq] (my 2048 tokens) -> Q^T
     duplicated on both partition halves (needed for row-packed scores).
     V^T chunks PE-transposed, masked, ones-column appended -> V' [128, 65]
     per chunk: the ones column makes PV also produce softmax denominators.
  2. Flash stage (qb, pr): two row-packed score matmuls (e=64 contraction
     in PE rows 0-63 / 64-127 concurrently) -> one [128, 1024] exp on
     ScalarE (scale 1/sqrt(64) folded) -> two PV matmuls accumulating
     out^T [65, 512] in PSUM.
  3. Scheduling: scores for stage s+1 are emitted before PV of stage s so
     ScalarE runs back-to-back; projection/transpose work is drip-fed as
     filler between stages. Three q-block accumulators live concurrently
     (qb0-2 woven with projections, qb3 after norm(0) frees a PSUM bank).
  4. Normalize: PE-transpose out^T chunks, DVE reciprocal of the sums
     column, multiply, one batched DMA out per q-block.
"""

import sys

if "/opt/trn_rl_repo" not in sys.path:
    sys.path.insert(0, "/opt/trn_rl_repo")

import ml_dtypes
import numpy as np

import concourse.bass as bass
import concourse.mybir as mybir
import concourse.tile as tile
from concourse.bass_utils import run_bass_kernel_spmd
from concourse.masks import make_identity

BF16 = mybir.dt.bfloat16
F32 = mybir.dt.float32
bf16 = ml_dtypes.bfloat16

B, S, D, E = 4, 4096, 1024, 64
SH = S // 2          # per-core query count
ND = D // 128        # d chunks
NK = S // 128        # key chunks
NTB = S // 512       # token blocks
NQB = SH // 512      # query blocks
NPR = NK // 2        # k-chunk pairs per query block
EV = E + 1           # V' columns (V | mask-ones)

LAST_EXEC_NS = None


def _split_multi_waits(nc, max_waits=1):
    """walrus in this container rejects instructions with >1 sync wait;
    hoist extra waits onto same-engine NOPs inserted just before."""
    for bb in nc.main_func.blocks:
        insts = bb.instructions
        out = []
        changed = False
        for inst in insts:
            si = inst.sync_info
            if si is not None and len(si.on_wait) > max_waits:
                waits = list(si.on_wait)
                extra, keep = waits[:-max_waits], waits[-max_waits:]
                for w in extra:
                    out.append(
                        mybir.InstNoOp(
                            name=nc.get_next_instruction_name(),
                            engine=inst.engine,
                            sync_info=mybir.SyncInfo(on_wait=[w], on_update=[]),
                        )
                    )
                inst.sync_info = mybir.SyncInfo(
                    on_wait=keep, on_update=list(si.on_update)
                )
                changed = True
            out.append(inst)
        if changed:
            bb.instructions = out
    return nc


def _build():
    nc = bass.Bass("TRN2", target_bir_lowering=False, debug=False, num_devices=8)

    # x^T host-swizzled: col = tb*4096 + d*512 + s maps to x[tb*512+s, d*128+p]
    xt_ext = nc.declare_dram_parameter("xt", [128, NTB * 4096], BF16, isOutput=False)
    # weights host-swizzled: [128, ND*128], w[p, d*128+j] = W[d*128+p, j]
    wvk_ext = nc.declare_dram_parameter("wvk", [128, ND * 128], BF16, isOutput=False)
    wqq_ext = nc.declare_dram_parameter("wqq", [128, ND * 128], BF16, isOutput=False)
    bvk_ext = nc.declare_dram_parameter("bvk", [128, 1], F32, isOutput=False)
    bqq_ext = nc.declare_dram_parameter("bqq", [128, 1], F32, isOutput=False)
    maskv_ext = nc.declare_dram_parameter("maskv", [128, NK], F32, isOutput=False)
    out_ext = nc.declare_dram_parameter("out", [SH, E], F32, isOutput=True)

    AT = mybir.ActivationFunctionType
    ALU = mybir.AluOpType

    with tile.TileContext(nc) as tc:
        with (
            tc.tile_pool(name="const", bufs=1) as cpool,
            tc.tile_pool(name="big", bufs=1) as bigpool,
            tc.tile_pool(name="work", bufs=3) as wpool,
            tc.tile_pool(name="nrm", bufs=2) as npool,
            tc.tile_pool(name="ps_a", bufs=1, space="PSUM") as ps_a,
            tc.tile_pool(name="ps_s", bufs=2, space="PSUM") as ps_s,
            tc.tile_pool(name="ps_o", bufs=3, space="PSUM") as ps_o,
        ):
            # ---- DMA dispatch order = critical path to tb0's projections.
            # x slabs alternate between the Sync and Scalar HWDGE queues so
            # two transfers stream in parallel ----
            xt_sb = bigpool.tile([128, NTB * 4096], BF16, tag="xt")
            nc.sync.dma_start(out=xt_sb[:, 0:2048], in_=xt_ext[:, 0:2048])
            nc.scalar.dma_start(out=xt_sb[:, 2048:4096], in_=xt_ext[:, 2048:4096])
            wvk_all = cpool.tile([128, ND * 128], BF16, tag="wvk")
            nc.sync.dma_start(out=wvk_all[:], in_=wvk_ext[:])
            bvk_sb = cpool.tile([128, 1], F32, tag="bvk")
            nc.sync.dma_start(out=bvk_sb[:], in_=bvk_ext[:])
            maskv_sb = cpool.tile([128, NK], F32, tag="maskv")
            nc.sync.dma_start(out=maskv_sb[:], in_=maskv_ext[:])
            wqq_all = cpool.tile([128, ND * 128], BF16, tag="wqq")
            nc.sync.dma_start(out=wqq_all[:], in_=wqq_ext[:])
            bqq_sb = cpool.tile([128, 1], F32, tag="bqq")
            nc.sync.dma_start(out=bqq_sb[:], in_=bqq_ext[:])
            for tb in range(1, NTB):
                eng = nc.sync if tb % 2 == 1 else nc.scalar
                eng.dma_start(
                    out=xt_sb[:, tb * 4096 : (tb + 1) * 4096],
                    in_=xt_ext[:, tb * 4096 : (tb + 1) * 4096],
                )

            wvk_sb = [wvk_all[:, d * 128 : (d + 1) * 128] for d in range(ND)]
            wqq_sb = [wqq_all[:, d * 128 : (d + 1) * 128] for d in range(ND)]
            id64 = cpool.tile([64, 64], BF16, tag="id64")
            make_identity(nc, id64[:])
            id65 = cpool.tile([65, 65], F32, tag="id65")
            make_identity(nc, id65[:])

            # ---- PE warm-up: dummy matmuls on the identity tile (no DMA
            # dependency) keep the HAM activity window busy during the x
            # slab0 wait so tb0's projections run at the warm 2.4 GHz ----
            warm_ps = ps_a.tile([64, 64], F32, tag="a", name="warm")
            for w in range(100):
                nc.tensor.matmul(
                    warm_ps[:],
                    id64[:],
                    id64[:],
                    start=True,
                    stop=True,
                    skip_group_check=True,
                )

            Q2 = bigpool.tile([128, SH], BF16, tag="q2")
            VK2T = bigpool.tile([128, S], BF16, tag="vk2t")  # V^T | K^T halves
            KD = bigpool.tile([64, S], BF16, tag="kd")       # K^T dup rows 0-63
            V_all = bigpool.tile([128, NK * EV], BF16, tag="vall")

            ones_col = V_all[:].rearrange("p (c e) -> p c e", e=EV)[:, :, E]
            nc.vector.tensor_copy(ones_col, maskv_sb[:])

            # ================= emission units =================
            # Filler units (projections, V' build, norms) are drip-fed
            # between flash stages to keep ScalarE saturated.
            fillers = []          # list of (required_before_marker, fn)
            proj_done_tb = [None] * NTB  # marker index per tb

            def mk_pass1(tb, dlo, dhi, ps_tile):
                def fn():
                    for d in range(dlo, dhi):
                        nc.tensor.matmul(
                            ps_tile[:],
                            wvk_sb[d],
                            xt_sb[:, tb * 4096 + d * 512 : tb * 4096 + (d + 1) * 512],
                            start=(d == 0),
                            stop=(d == ND - 1),
                            skip_group_check=True,
                        )
                return fn

            def mk_pass1_bias(tb, ps_tile):
                sl = slice(tb * 512, (tb + 1) * 512)
                def fn():
                    nc.vector.tensor_scalar(
                        VK2T[:, sl], ps_tile[:], bvk_sb[:], None, ALU.add
                    )
                    # K^T dup onto partitions 0-63 (DVE copy, off the DMA queue)
                    nc.vector.tensor_copy(KD[:, sl], VK2T[64:128, sl])
                return fn

            def mk_pass2(tb, dlo, dhi, ps_tile):
                def fn():
                    for d in range(dlo, dhi):
                        nc.tensor.matmul(
                            ps_tile[:],
                            wqq_sb[d],
                            xt_sb[:, tb * 4096 + d * 512 : tb * 4096 + (d + 1) * 512],
                            start=(d == 0),
                            stop=(d == ND - 1),
                            skip_group_check=True,
                        )
                return fn

            def mk_pass2_bias(tb, ps_tile):
                sl = slice(tb * 512, (tb + 1) * 512)
                def fn():
                    nc.vector.tensor_scalar(
                        Q2[:, sl], ps_tile[:], bqq_sb[:], None, ALU.add
                    )
                return fn

            def mk_vchunk(c):
                def fn():
                    psv = ps_a.tile([128, 64], BF16, tag="a")
                    nc.tensor.transpose(
                        psv[:], VK2T[0:64, c * 128 : (c + 1) * 128], id64[:]
                    )
                    nc.vector.tensor_scalar(
                        V_all[:, c * EV : c * EV + E],
                        psv[:],
                        maskv_sb[:, c : c + 1],
                        None,
                        ALU.mult,
                    )
                return fn

            for tb in range(NTB):
                ps1 = ps_a.tile([128, 512], F32, tag="a", name=f"p1_{tb}")
                for dlo in range(0, ND, 2):
                    fillers.append((tb, mk_pass1(tb, dlo, dlo + 2, ps1)))
                fillers.append((tb, mk_pass1_bias(tb, ps1)))
                if tb < NQB:
                    ps2 = ps_a.tile([128, 512], F32, tag="a", name=f"p2_{tb}")
                    for dlo in range(0, ND, 2):
                        fillers.append((tb, mk_pass2(tb, dlo, dlo + 2, ps2)))
                    fillers.append((tb, mk_pass2_bias(tb, ps2)))
                for c in range(tb * 4, tb * 4 + 4):
                    fillers.append((tb, mk_vchunk(c)))
                proj_done_tb[tb] = len(fillers)

            # tile allocation order fixes PSUM slots; allocate lazily via dict
            pso_tiles = {}

            def get_pso(qb):
                if qb not in pso_tiles:
                    pso_tiles[qb] = ps_o.tile([EV, 512], F32, tag="o", name=f"pso{qb}")
                return pso_tiles[qb]

            s2_of = {}

            def emit_scores(pr, qb):
                qsl = slice(qb * 512, (qb + 1) * 512)
                kA, kB = 2 * pr, 2 * pr + 1
                S2 = ps_s.tile([128, 1024], F32, tag="s", name=f"s2_{qb}_{pr}")
                s2_of[(pr, qb)] = S2
                nc.tensor.matmul(
                    S2[:, 0:512],
                    KD[:, kA * 128 : (kA + 1) * 128],
                    Q2[0:64, qsl],
                    start=True,
                    stop=True,
                )
                nc.tensor.matmul(
                    S2[:, 512:1024],
                    VK2T[64:128, kB * 128 : (kB + 1) * 128],
                    Q2[64:128, qsl],
                    start=True,
                    stop=True,
                )

            pt_of = {}

            def emit_exp(pr, qb, split=False):
                S2 = s2_of[(pr, qb)]
                PT = wpool.tile([128, 1024], BF16, tag="pt", bufs=4)
                pt_of[(pr, qb)] = PT
                if split:  # last stage: halve ACT latency on the tail
                    nc.scalar.activation(
                        PT[:, 0:512], S2[:, 0:512], AT.Exp, bias=0.0, scale=0.125
                    )
                    nc.scalar.activation(
                        PT[:, 512:1024], S2[:, 512:1024], AT.Exp, bias=0.0, scale=0.125
                    )
                else:
                    nc.scalar.activation(PT[:], S2[:], AT.Exp, bias=0.0, scale=0.125)

            def emit_pv(pr, qb):
                pso = get_pso(qb)
                PT = pt_of.pop((pr, qb))
                kA, kB = 2 * pr, 2 * pr + 1
                nc.tensor.matmul(
                    pso[:],
                    V_all[:, kA * EV : (kA + 1) * EV],
                    PT[:, 0:512],
                    start=(pr == 0),
                    stop=False,
                    skip_group_check=True,
                )
                nc.tensor.matmul(
                    pso[:],
                    V_all[:, kB * EV : (kB + 1) * EV],
                    PT[:, 512:1024],
                    start=False,
                    stop=(pr == NPR - 1),
                    skip_group_check=True,
                )

            def mk_norm_units(qb):
                # single [128, 4*65] PSUM tile: all 4 transposes batched (no
                # PE/DVE ping-pong on the shared ps_a bank), then recip+mul
                pso = pso_tiles[qb]
                t_out = npool.tile([EV, 512], F32, tag="tout", name=f"to{qb}")
                osb = npool.tile([128, 4 * E], F32, tag="osb", name=f"osb{qb}")
                ptn = ps_a.tile([128, 4 * EV], F32, tag="a", name=f"ptn{qb}")
                units = []
                units.append(lambda: nc.vector.tensor_copy(t_out[:], pso[:]))

                def mk_transp(c0):
                    def fn():
                        for c in (c0, c0 + 1):
                            nc.tensor.transpose(
                                ptn[:, c * EV : (c + 1) * EV],
                                t_out[:, c * 128 : (c + 1) * 128],
                                id65[:],
                            )
                    return fn

                def mk_nrm(c0):
                    def fn():
                        for c in (c0, c0 + 1):
                            recip = npool.tile([128, 1], F32, tag="recip")
                            nc.vector.reciprocal(
                                recip[:], ptn[:, c * EV + E : c * EV + E + 1]
                            )
                            nc.vector.tensor_scalar(
                                osb[:, c * E : (c + 1) * E],
                                ptn[:, c * EV : c * EV + E],
                                recip[:],
                                None,
                                ALU.mult,
                            )
                    return fn

                units += [mk_transp(0), mk_transp(2), mk_nrm(0), mk_nrm(2)]

                def out_dma():
                    src = osb[:].rearrange("p (c e) -> p c e", e=E)
                    dst = out_ext[qb * 512 : (qb + 1) * 512, :].rearrange(
                        "(c p) e -> p c e", p=128
                    )
                    nc.sync.dma_start(out=dst, in_=src)

                units.append(out_dma)
                return units

            # ---- stage order: qb0-2 woven with projections, then qb3 ----
            stages = []
            for t in range(NTB):
                for qb in (0, 1, 2):
                    if t == qb:
                        stages += [(p, qb) for p in range(0, 2 * t)]
                    if qb <= t:
                        stages += [(2 * t, qb), (2 * t + 1, qb)]
            for p in range(NPR):
                stages.append((p, 3))

            # required filler progress before stage (pr, qb) can run
            def req_marker(pr, qb):
                tb_need = max((2 * pr + 1) // 4, min(qb, NQB - 1))
                return proj_done_tb[tb_need]

            # ---- main emission loop ----
            fcursor = 0

            def drain_to(m):
                nonlocal fcursor
                while fcursor < m:
                    fillers[fcursor][1]()
                    fcursor += 1

            def fill(n, stage_idx):
                # only drip-feed proj units whose x slab has surely landed
                # (slab tb arrives ~4.3+2.9(tb+1)us; stage i runs ~10+1.15i)
                nonlocal fcursor
                e = min(fcursor + n, len(fillers))
                while fcursor < e:
                    tb = fillers[fcursor][0]
                    if 2.9 * tb > 2.8 + 1.15 * stage_idx:
                        break
                    fillers[fcursor][1]()
                    fcursor += 1

            prev = None
            norm_queue = []
            done_count = {0: 0, 1: 0, 2: 0, 3: 0}
            for i, s in enumerate(stages):
                drain_to(req_marker(*s))
                emit_scores(*s)
                emit_exp(*s, split=(i == len(stages) - 1))
                fill(1, i)
                if norm_queue:
                    norm_queue.pop(0)()
                if prev is not None:
                    emit_pv(*prev)
                    done_count[prev[1]] += 1
                    if done_count[prev[1]] == NPR and prev[1] < 3:
                        norm_queue += mk_norm_units(prev[1])
                fill(1, i)
                prev = s
            emit_pv(*prev)
            for u in norm_queue:
                u()
            for u in mk_norm_units(3):
                u()

    _split_multi_waits(nc)
    return nc


_NC_CACHE = [None]


def kernel(x, mask, Wq, bq, Wk, bk, Wv, bv, _trace=False, _tmpdir=None):
    global LAST_EXEC_NS
    x = np.asarray(x, dtype=np.float32)
    mask = np.asarray(mask)
    Wq, bq = np.asarray(Wq, np.float32), np.asarray(bq, np.float32)
    Wk, bk = np.asarray(Wk, np.float32), np.asarray(bk, np.float32)
    Wv, bv = np.asarray(Wv, np.float32), np.asarray(bv, np.float32)

    def swz(w):  # [D, 128] -> [128, ND*128]: out[p, d*128+j] = w[d*128+p, j]
        return np.ascontiguousarray(
            w.reshape(ND, 128, 128).transpose(1, 0, 2).reshape(128, ND * 128)
        ).astype(bf16)

    wvk = swz(np.concatenate([Wv, Wk], axis=1))
    wqq = swz(np.concatenate([Wq, Wq], axis=1))
    bvk = np.concatenate([bv, bk])[:, None].astype(np.float32)
    bqq = np.concatenate([bq, bq])[:, None].astype(np.float32)

    in_maps = []
    for c in range(8):
        b, h = c // 2, c % 2
        xb = x[b]  # [S, D]
        mb = mask[b].astype(np.float32)  # [S]
        if h == 1:  # my query tokens first
            order = np.concatenate([np.arange(SH, S), np.arange(0, SH)])
            xb = xb[order]
            mb = mb[order]
        # xt[p, tb*4096 + d*512 + s] = xb[tb*512+s, d*128+p]
        xt = np.ascontiguousarray(
            xb.reshape(NTB, 512, ND, 128).transpose(3, 0, 2, 1).reshape(128, -1)
        ).astype(bf16)
        maskv = np.ascontiguousarray(mb.reshape(NK, 128).T).astype(np.float32)
        in_maps.append(
            {
                "xt": xt,
                "wvk": wvk,
                "wqq": wqq,
                "bvk": bvk,
                "bqq": bqq,
                "maskv": maskv,
            }
        )

    if _NC_CACHE[0] is None:
        _NC_CACHE[0] = _build()
    nc = _NC_CACHE[0]

    kwargs = {}
    if _trace:
        kwargs = dict(trace=True, tmpdir=_tmpdir)
    res = run_bass_kernel_spmd(nc, in_maps, list(range(8)), **kwargs)
    LAST_EXEC_NS = res.exec_time_ns

    out = np.empty((B, S, E), dtype=np.float32)
    for c in range(8):
        b, h = c // 2, c % 2
        out[b, h * SH : (h + 1) * SH, :] = res.results[c]["out"]
    return out
